# revision 24
# baseline (speedup 1.0000x reference)
"""GQA attention kernel for Trainium2, tensor-parallel over 8 NeuronCores.

Sharding: 4 q-heads + 1 kv-group per core (H=32, G=8). Each core computes
its heads' attention and a partial out-projection; host sums the 8 partials
(the "all-reduce after out_proj").

Device layout strategy: everything transposed (head_dim on partitions,
sequence on the free dim) so Q/K projections, scores and ctx matmuls all
run at moving-dim 512, and no runtime transposes are needed except V
(16 small PE transposes). All heavy matmuls run in bf16 (fp32 PSUM
accumulation): bf16 streams 1 elem/cycle through the PE array vs ~2
cycles for fp32, and LDWEIGHTS gets fast-weight-load.

RMSNorm over head_dim (= partitions) is a ones-matmul partition reduction
for all 5 rows of a chunk (2 q-pack pairs + k) accumulated into ONE [5,512]
PSUM tile, a 5-instruction DVE quake-rsqrt chain (bit-trick seed + one
Newton step; bf16 output floor dominates the seed error), and [5,*]-weight
broadcast-matmuls back to 128/64 partitions. The scalar engine therefore
only ever executes Exp (+Copy), so the ACT table is loaded exactly once —
the Ln/Exp/Reciprocal table-swap thrash (48 x 1.28us) is gone.

RoPE's rotate_half is a 32-lane pairwise stream_shuffle because the
head-dim components are permuted host-side into (i, i+32) pairs
(cos/sin/gains/wq/wk rows permuted to match; wo is NOT permuted since
V/ctx stay in natural order).

Causal mask: scores_masked = min(BIG*(q+0.5) - BIG*(k), s) applied ONLY to
the 128 partially-masked diagonal columns of diagonal key-blocks, for both
heads of a pack in one [128,2,128] scalar_tensor_tensor. exp(0.125*masked)
== 0. Fully-masked column ranges (local q < 128r) are skipped entirely:
the scores matmul, exp and ctx matmul all narrow to columns [128r:512].

exp runs on the scalar engine over both heads of a pack at once (the two
heads' score blocks live in one 2-bank PSUM tile), halving per-instruction
overhead. Softmax denominators ride the ctx matmul as an appended
ones-column on V; reciprocals are one batched DVE reciprocal_approx_fast
over a [16,512] tile of all chunk denominators.
"""

import sys
from contextlib import ExitStack

import numpy as np

for _p in ("/opt/trn_rl_repo",):
    if _p not in sys.path:
        sys.path.insert(0, _p)

import ml_dtypes

import concourse.bass as bass
import concourse.tile as tile
from concourse import bacc, mybir
from concourse.bass_utils import run_bass_kernel_spmd

F32 = mybir.dt.float32
BF16 = mybir.dt.bfloat16
U32 = mybir.dt.uint32
AF = mybir.ActivationFunctionType
ALU = mybir.AluOpType

B, S, D = 1, 2048, 2048
H, G, HD = 32, 8, 64
NCORES = 8
HPC = H // NCORES          # 4 q heads per core
BIG = 1000.0
SCW = 512                  # s-chunk width (matmul moving dim)
KBW = 128                  # key block width
LN2 = 0.6931471805599453

# head-dim permutation: new 2j <- old j, new 2j+1 <- old j+32
PERM = np.empty(64, dtype=np.int64)
PERM[0::2] = np.arange(32)
PERM[1::2] = np.arange(32) + 32
PARTNER = np.empty(64, dtype=np.int64)
PARTNER[0::2] = np.arange(1, 64, 2)
PARTNER[1::2] = np.arange(0, 64, 2)
SIGN = np.empty(64, dtype=np.float32)
SIGN[0::2] = -1.0
SIGN[1::2] = 1.0

_SHUF_MASK = [i + 1 if i % 2 == 0 else i - 1 for i in range(32)]


def _rope(nc, pools, psC, bc_ps, n_rows, cg_t, sg_t, out_ap, sc):
    """RoPE + rms-scale on a normalized-projection chunk in SBUF.

    psC: (n_rows, SCW) SBUF f32 raw projected values (d on partitions).
    bc_ps: (n_rows, SCW) PSUM f32 broadcast 1/rms values.
    cg_t/sg_t: (n_rows, S) SBUF cos*g / +-sin*g_partner tables.
    out_ap: destination SBUF AP (n_rows, SCW) bf16.
    """
    tmp = pools["tmp"]
    sl = bass.ts(sc, SCW)
    shuf = tmp.tile([n_rows, SCW], F32, tag="shuf")
    nc.vector.stream_shuffle(shuf, psC, mask=_SHUF_MASK)
    t1 = tmp.tile([n_rows, SCW], F32, tag="t1")
    nc.vector.tensor_mul(t1, psC, cg_t[:, sl])
    t2 = tmp.tile([n_rows, SCW], F32, tag="t2")
    nc.vector.tensor_mul(t2, shuf, sg_t[:, sl])
    t3 = tmp.tile([n_rows, SCW], F32, tag="t3")
    nc.gpsimd.tensor_add(t3, t1, t2)
    nc.vector.tensor_mul(out_ap, t3, bc_ps)


def _build(nc):
    SC = S // SCW
    KB = S // KBW
    DT = D // 128

    B_NAMES = {"xT", "wqT", "wkvT", "woT", "hs5", "bc5", "bcz", "ident"}
    dt_in = {}
    for name, shape in [
        ("xT", [D, S]), ("wqT", [D, 2 * 128]), ("wkvT", [D, 128]),
        ("woT", [2 * 128, D]), ("cgq", [64, S]), ("sgq", [64, S]),
        ("cgk", [64, S]), ("sgk", [64, S]),
        ("hs5", [128, 3, 5]), ("bc5", [5, 3, 128]),
        ("iotaP", [128, 4, 2, 128]), ("thrB", [128, SCW // KBW]),
        ("ident", [64, 64]),
    ]:
        dt_in[name] = nc.dram_tensor(
            name, shape,
            BF16 if name in B_NAMES else F32,
            kind="ExternalInput").ap()
    y_dram = nc.dram_tensor("y", [S, D], BF16, kind="ExternalOutput").ap()

    with tile.TileContext(nc) as tc, ExitStack() as ctx:
        ctx.enter_context(nc.allow_low_precision(
            reason="bf16 matmul operands; fp32 PSUM accumulation throughout"))
        consts = ctx.enter_context(tc.tile_pool(name="consts", bufs=1))
        persist = ctx.enter_context(tc.tile_pool(name="persist", bufs=1))
        tmp = ctx.enter_context(tc.tile_pool(name="tmp", bufs=2))

        # ---- constants / weights into SBUF ----
        def load(name, shape=None, double=False):
            ap = dt_in[name]
            shape = shape or list(ap.shape)
            t = consts.tile(shape, ap.dtype, tag=name, name=name)
            if double:
                nc.sync.dma_start(t[0:64], ap)
                nc.sync.dma_start(t[64:128], ap)
            else:
                nc.sync.dma_start(t, ap.rearrange("(t p) s -> p t s", p=128)
                                  if len(shape) == 3 and name in ("wqT", "wkvT", "woT")
                                  else ap)
            return t

        # projection weights first — phase 1's matmuls gate everything.
        wq_t = load("wqT", [128, DT, 256])
        wkv_t = load("wkvT", [128, DT, 128])


        qrt = [persist.tile([128, S], BF16, tag=f"qrt{p}", name=f"qrt{p}") for p in range(2)]
        krt = persist.tile([128, S], BF16, tag="krt")
        vt = persist.tile([64, S], BF16, tag="vt")
        vaug = persist.tile([128, KB, 65], BF16, tag="vaug")
        ctxT = [persist.tile([128, S], BF16, tag=f"ctxT{p}", name=f"ctxT{p}") for p in range(2)]
        zall = persist.tile([1, 16 * SCW], F32, tag="zall")

        xT_r = dt_in["xT"].rearrange("(t p) s -> p t s", p=128)

        # ---- fully fused: projections + norm/rope + attention + out-proj ----
        # PSUM (8 banks): proj 1x1 + sm 1x1 + sps 2x1 + cps 1x2 + yps 1x2 = 8.
        # Attention for chunk jc=sc runs right after chunk sc's projections;
        # out-projection work units of chunk jc-1 are woven into chunk jc's
        # attention kb loop so the PE fills attention's exp-wait bubbles.
        with tc.tile_pool(name="xin", bufs=16) as xin, \
             tc.tile_pool(name="projps", bufs=1, space="PSUM") as projps, \
             tc.tile_pool(name="smallps", bufs=1, space="PSUM") as smallps, \
             tc.tile_pool(name="sps", bufs=1, space="PSUM") as spsp, \
             tc.tile_pool(name="cps", bufs=1, space="PSUM") as cpsp, \
             tc.tile_pool(name="yps", bufs=2, space="PSUM") as ypsp, \
             tc.tile_pool(name="epool", bufs=3) as epool, \
             tc.tile_pool(name="yout", bufs=3) as youtp:
            pools = {"tmp": tmp, "smallps": smallps}

            yts = {}

            def outproj_units(jc):
                """Out-projection of chunk jc as a list of work-unit
                closures, each one (sb, dc) pair of accumulating matmuls +
                drain copy, plus one DMA unit per finished s-block."""
                units = []
                for sb in range(4 * jc, 4 * jc + 4):
                    def start_sb(sb=sb):
                        yts[sb] = youtp.tile([128, D], BF16, tag="yt",
                                             name=f"yt{sb}")
                    for dc in range(D // SCW):
                        def unit(sb=sb, dc=dc, first=(dc == 0)):
                            if first:
                                start_sb(sb)
                            yt = yts[sb]
                            yps = ypsp.tile([128, SCW], F32, tag="yps",
                                            name=f"yps{sb}_{dc}")
                            for p in range(2):
                                nc.tensor.matmul(
                                    yps, ctxT[p][:, bass.ts(sb, 128)],
                                    wo_t[:, p, bass.ts(dc, SCW)],
                                    start=(p == 0), stop=(p == 1))
                            if dc % 2 == 0:
                                nc.vector.tensor_copy(
                                    out=yt[:, bass.ts(dc, SCW)], in_=yps)
                            else:
                                nc.scalar.copy(yt[:, bass.ts(dc, SCW)], yps)
                            if dc == D // SCW - 1:
                                nc.sync.dma_start(
                                    y_dram[bass.ts(sb, 128), :], yt)
                        units.append(unit)
                return units

            pending = []
            for sc in range(SC):
                if sc == 0:
                    # emit x-chunk DMAs for sc=0 before the table loads so the
                    # first projection matmuls start as early as possible.
                    xts0 = []
                    for dt2 in range(DT // 2):
                        xt = xin.tile([128, 2, SCW], BF16, tag="xt",
                                      name=f"xt0_{dt2}")
                        nc.sync.dma_start(
                            xt, xT_r[:, 2 * dt2:2 * dt2 + 2, bass.ts(0, SCW)])
                        xts0.append(xt)
                    # rope/norm tables — not needed until the first norm.
                    cgq_t = load("cgq", [128, S], double=True)
                    sgq_t = load("sgq", [128, S], double=True)
                    cgk_t = load("cgk")
                    sgk_t = load("sgk")
                    hs5_t = load("hs5")
                    bc5_t = load("bc5")
                    ident_t = load("ident")
                    iota_t = load("iotaP"); thr_t = load("thrB")
                    ones_t = consts.tile([128, KB], BF16, tag="ones",
                                         name="ones")
                    nc.vector.memset(ones_t, 1.0)
                    onesw = consts.tile([1, 64], BF16, tag="onesw",
                                        name="onesw")
                    nc.vector.memset(onesw, 1.0)
                    nc.vector.tensor_copy(
                        out=vaug[:, :, 64:65],
                        in_=ones_t.rearrange("p (k o) -> p k o", o=1))
                # x-chunk tiles up front; the three projection targets run
                # sequentially through ONE rotating PSUM bank (the bank
                # frees on each target's psC drain copy).
                if sc == 0:
                    xts = xts0
                else:
                    xts = []
                    for dt2 in range(DT // 2):
                        xt = xin.tile([128, 2, SCW], BF16, tag="xt",
                                      name=f"xt{sc}_{dt2}")
                        nc.sync.dma_start(
                            xt, xT_r[:, 2 * dt2:2 * dt2 + 2, bass.ts(sc, SCW)])
                        xts.append(xt)

                # ---- norm: accumulate all 5 mean-square rows into rms5 ----
                rms5 = smallps.tile([5, SCW], F32, tag="sm", name=f"rms5_{sc}")
                psCs = []
                for i, (w_t, cols) in enumerate(
                        [(wq_t, slice(0, 128)), (wq_t, slice(128, 256)),
                         (wkv_t, slice(0, 128))]):
                    ps = projps.tile([128, SCW], F32, tag="proj",
                                     name=f"proj{sc}_{i}")
                    for dt2 in range(DT // 2):
                        for j in range(2):
                            nc.tensor.matmul(
                                ps, w_t[:, 2 * dt2 + j, cols],
                                xts[dt2][:, j, :],
                                start=(dt2 == 0 and j == 0),
                                stop=(dt2 == DT // 2 - 1 and j == 1))
                    if i < 2:
                        psC = tmp.tile([128, SCW], F32, tag="psC",
                                       name=f"psC{i}_{sc}")
                        nc.vector.tensor_copy(out=psC, in_=ps)
                        sq = tmp.tile([128, SCW], BF16, tag="sq")
                        nc.vector.tensor_mul(sq, psC, psC)
                        nc.tensor.matmul(rms5, hs5_t[:, i, :], sq,
                                         start=(i == 0), stop=False)
                        psCs.append(psC)
                    else:
                        nc.vector.tensor_copy(out=vt[:, bass.ts(sc, SCW)],
                                              in_=ps[64:128, :])
                        psCk = tmp.tile([64, SCW], F32, tag="psC",
                                        name=f"psCk_{sc}")
                        nc.vector.tensor_copy(out=psCk, in_=ps[0:64, :])
                        sqk = tmp.tile([64, SCW], BF16, tag="sq")
                        nc.vector.tensor_mul(sqk, psCk, psCk)
                        nc.tensor.matmul(rms5, hs5_t[0:64, 2, :], sqk,
                                         start=False, stop=True)

                # ---- rsqrt chain: rcp5 = 1/sqrt(rms5) ----
                # seed = exp(-0.5*ln~(m)) where ln~ is the bit-trick linear
                # log (u32 bits numerically cast to f32, one mul-add); the
                # exp runs on ACT whose table is already resident from the
                # attention exps (zero table loads). One Newton step on DVE
                # polishes to ~1.4e-3, under the bf16 output floor.
                tf5 = tmp.tile([5, SCW], F32, tag="tf5", bufs=1)
                nc.vector.tensor_copy(out=tf5, in_=rms5.bitcast(U32))
                nc.vector.tensor_scalar(
                    out=tf5, in0=tf5, scalar1=float(-0.5 * LN2 / 2.0 ** 23),
                    scalar2=float(0.5 * LN2 * 127.0), op0=ALU.mult,
                    op1=ALU.add)
                y5 = tmp.tile([5, SCW], F32, tag="y5", bufs=1)
                nc.scalar.activation(y5, tf5, AF.Exp)
                a5 = tmp.tile([5, SCW], F32, tag="a5", bufs=1)
                nc.vector.tensor_mul(a5, y5, y5)
                nc.vector.scalar_tensor_tensor(
                    out=a5, in0=a5, scalar=-0.5, in1=rms5,
                    op0=ALU.mult, op1=ALU.mult)
                rcp5 = tmp.tile([5, SCW], BF16, tag="rcp5")
                nc.vector.scalar_tensor_tensor(
                    out=rcp5, in0=a5, scalar=1.5, in1=y5,
                    op0=ALU.add, op1=ALU.mult)

                # ---- broadcast 1/rms + rope ----
                for p in range(2):
                    bc_ps = smallps.tile([128, SCW], F32, tag="sm",
                                         name=f"bcq{p}_{sc}")
                    nc.tensor.matmul(bc_ps, bc5_t[:, p, :], rcp5,
                                     start=True, stop=True)
                    _rope(nc, pools, psCs[p], bc_ps, 128, cgq_t, sgq_t,
                          qrt[p][:, bass.ts(sc, SCW)], sc)
                bc_psk = smallps.tile([64, SCW], F32, tag="sm",
                                      name=f"bck_{sc}")
                nc.tensor.matmul(bc_psk, bc5_t[:, 2, 0:64], rcp5,
                                 start=True, stop=True)
                _rope(nc, pools, psCk, bc_psk, 64, cgk_t, sgk_t,
                      krt[0:64, bass.ts(sc, SCW)], sc)
                # duplicate roped K into rows 64-127 for row-tiled paired scores
                nc.vector.tensor_copy(out=krt[64:128, bass.ts(sc, SCW)],
                                      in_=krt[0:64, bass.ts(sc, SCW)])
                # V transposes for this chunk (tps shares the sm slot family)
                for t in range(sc * 4, sc * 4 + 4):
                    tps = smallps.tile([128, 64], BF16, tag="sm",
                                       name=f"tps{t}")
                    nc.tensor.transpose(tps, vt[:, bass.ts(t, KBW)], ident_t)
                    nc.vector.tensor_copy(out=vaug[:, t, 0:64], in_=tps)
                if sc == 0:
                    wo_t = load("woT", [128, 2, D])  # [p,j,:]=woT[128j+p,:]

                # ---- attention for jc = sc, outproj(jc-1) units woven in ----
                jc = sc
                nkb = (SCW * (jc + 1)) // KBW
                for pack in range(2):
                    cps = [cpsp.tile([65, SCW], F32, tag=f"cps{h}",
                                     name=f"cps{jc}_{pack}_{h}") for h in range(2)]
                    for kb in range(nkb):
                        r = kb - (SCW // KBW) * jc
                        q0 = 128 * r if r > 0 else 0   # first live local column
                        sps = spsp.tile([128, 2, SCW], F32, tag="sps",
                                        name=f"sps{jc}_{pack}_{kb}")
                        for h in range(2):
                            b = 64 * h
                            nc.tensor.matmul(
                                sps[:, h, q0:], krt[b:b + 64, bass.ts(kb, KBW)],
                                qrt[pack][b:b + 64, jc * SCW + q0:(jc + 1) * SCW],
                                start=True, stop=True, tile_position=(b, 0))
                        if r >= 0:
                            # mask only the 128 partially-masked diagonal
                            # columns, both heads in one op
                            nc.vector.scalar_tensor_tensor(
                                out=sps[:, :, q0:q0 + KBW], in0=iota_t[:, r],
                                scalar=thr_t[:, r:r + 1],
                                in1=sps[:, :, q0:q0 + KBW],
                                op0=ALU.subtract, op1=ALU.min)
                        et = epool.tile([128, 2, SCW], BF16, tag="et")
                        nc.scalar.activation(et[:, :, q0:], sps[:, :, q0:],
                                             AF.Exp, scale=HD ** -0.5)
                        for h in range(2):
                            nc.tensor.matmul(
                                cps[h][:, q0:], vaug[:, kb, :], et[:, h, q0:],
                                start=(kb == 0), stop=(kb == nkb - 1))
                        if pending:
                            pending.pop(0)()
                    # normalize this pack's ctx straight out of PSUM:
                    # 1/z on DVE (no tables), PE broadcast, one mul per head
                    zb = (jc * 2 + pack) * 2
                    for h in range(2):
                        nc.vector.tensor_copy(
                            out=zall[:, bass.ts(zb + h, SCW)],
                            in_=cps[h][64:65, :])
                    zsl = slice(zb * SCW, (zb + 2) * SCW)
                    nc.vector.reciprocal_approx_fast(out=zall[:, zsl],
                                                     in_=zall[:, zsl])
                    zrcp = tmp.tile([1, 2 * SCW], BF16, tag="zrcp",
                                    name=f"zrcp{jc}_{pack}")
                    nc.vector.tensor_copy(out=zrcp, in_=zall[:, zsl])
                    for h in range(2):
                        ctxS = tmp.tile([64, SCW], F32, tag="ctxS",
                                        name=f"ctxS{jc}_{pack}_{h}")
                        nc.vector.tensor_copy(out=ctxS, in_=cps[h][0:64, :])
                        bc_ps = smallps.tile([64, SCW], F32, tag="sm",
                                             name=f"zbc{jc}_{pack}_{h}")
                        nc.tensor.matmul(bc_ps, onesw,
                                         zrcp[:, bass.ts(h, SCW)],
                                         start=True, stop=True)
                        nc.vector.tensor_mul(
                            ctxT[pack][64 * h:64 * h + 64, bass.ts(jc, SCW)],
                            ctxS, bc_ps)
                pending.extend(outproj_units(jc))
            # drain the last chunk's out-projection
            for unit in pending:
                unit()


_CACHE = {}


def _get_nc():
    if "nc" not in _CACHE:
        nc = bacc.Bacc("TRN2", target_bir_lowering=False, debug=False)
        _build(nc)
        nc.compile()
        _CACHE["nc"] = nc
    return _CACHE["nc"]


def _prep_core_inputs(c, x, cos, sin, wq, wk, wv, wo, qg, kg):
    """Host-side sharding + layout marshaling for core c."""
    f = np.float32
    bf = ml_dtypes.bfloat16
    hsl = slice(c * HPC * HD, (c + 1) * HPC * HD)
    gsl = slice(c * HD, (c + 1) * HD)

    wq_c = wq[hsl, :].reshape(HPC, HD, D)[:, PERM, :].reshape(HPC * HD, D)
    wk_c = wk[gsl, :][PERM, :]
    wv_c = wv[gsl, :]

    cos_p, sin_p = cos[:, PERM], sin[:, PERM]
    qg_p, kg_p = qg[PERM], kg[PERM]

    cgq = np.ascontiguousarray((cos_p * qg_p).T, dtype=f)
    sgq = np.ascontiguousarray((sin_p * (SIGN * qg_p[PARTNER])).T, dtype=f)
    cgk = np.ascontiguousarray((cos_p * kg_p).T, dtype=f)
    sgk = np.ascontiguousarray((sin_p * (SIGN * kg_p[PARTNER])).T, dtype=f)

    # mean-square reduction weights: rms5 row layout
    # rows 0-1: q pack0 heads, rows 2-3: q pack1 heads, row 4: k
    hs5 = np.zeros((128, 3, 5), f)
    hs5[0:64, 0, 0] = 1.0 / HD
    hs5[64:128, 0, 1] = 1.0 / HD
    hs5[0:64, 1, 2] = 1.0 / HD
    hs5[64:128, 1, 3] = 1.0 / HD
    hs5[0:64, 2, 4] = 1.0 / HD
    # broadcast-back weights
    bc5 = np.zeros((5, 3, 128), f)
    bc5[0, 0, 0:64] = 1.0
    bc5[1, 0, 64:128] = 1.0
    bc5[2, 1, 0:64] = 1.0
    bc5[3, 1, 64:128] = 1.0
    bc5[4, 2, 0:64] = 1.0
    iotaP = np.empty((128, 4, 2, 128), f)
    for r in range(4):
        iotaP[:, r, :, :] = ((128 * r + np.arange(128, dtype=f) + 0.5)
                             * BIG)[None, None, :]
    thrB = np.ascontiguousarray(
        (np.arange(128, dtype=f)[:, None]
         + KBW * np.arange(SCW // KBW, dtype=f)[None, :]) * BIG, f)

    return {
        "xT": np.ascontiguousarray(x[0].T).astype(bf),
        "wqT": np.ascontiguousarray(wq_c.T).astype(bf),
        "wkvT": np.ascontiguousarray(np.concatenate([wk_c, wv_c], 0).T).astype(bf),
        "woT": np.ascontiguousarray(wo[:, hsl].T).astype(bf),
        "cgq": cgq, "sgq": sgq, "cgk": cgk, "sgk": sgk,
        "hs5": hs5.astype(bf), "bc5": bc5.astype(bf),
        "iotaP": iotaP, "thrB": thrB,
        "ident": np.eye(64, dtype=f).astype(bf),
    }


def kernel(x, mask, cos, sin, wq, wk, wv, wo, qg, kg, _trace=False):
    nc = _get_nc()
    in_maps = [
        _prep_core_inputs(c, np.asarray(x), np.asarray(cos), np.asarray(sin),
                          np.asarray(wq), np.asarray(wk), np.asarray(wv),
                          np.asarray(wo), np.asarray(qg), np.asarray(kg))
        for c in range(NCORES)
    ]
    res = run_bass_kernel_spmd(nc, in_maps, core_ids=list(range(NCORES)),
                               trace=_trace)
    _CACHE["last_results"] = res
    partials = np.stack([np.asarray(r["y"], dtype=np.float64)
                         for r in res.results], axis=0)
    out = partials.sum(axis=0).astype(np.float32)
    return out.reshape(B, S, D)


# revision 30
# speedup vs baseline: 1.2626x; 1.2626x over previous
"""GQA attention kernel for Trainium2, tensor-parallel over 8 NeuronCores.

Sharding: 4 q-heads + 1 kv-group per core (H=32, G=8). Each core computes
its heads' attention and a partial out-projection; host sums the 8 partials
(the "all-reduce after out_proj").

Device layout strategy: everything transposed (head_dim on partitions,
sequence on the free dim) so Q/K projections, scores and ctx matmuls all
run at moving-dim 512, and no runtime transposes are needed except V
(16 small PE transposes). All heavy matmuls run in bf16 (fp32 PSUM
accumulation): bf16 streams 1 elem/cycle through the PE array vs ~2
cycles for fp32, and LDWEIGHTS gets fast-weight-load.

RMSNorm over head_dim (= partitions) is a ones-matmul partition reduction
for all 5 rows of a chunk (2 q-pack pairs + k) accumulated into ONE [5,512]
PSUM tile, a 5-instruction DVE quake-rsqrt chain (bit-trick seed + one
Newton step; bf16 output floor dominates the seed error), and [5,*]-weight
broadcast-matmuls back to 128/64 partitions. The scalar engine therefore
only ever executes Exp (+Copy), so the ACT table is loaded exactly once —
the Ln/Exp/Reciprocal table-swap thrash (48 x 1.28us) is gone.

RoPE's rotate_half is a 32-lane pairwise stream_shuffle because the
head-dim components are permuted host-side into (i, i+32) pairs
(cos/sin/gains/wq/wk rows permuted to match; wo is NOT permuted since
V/ctx stay in natural order).

Causal mask: scores_masked = min(BIG*(q+0.5) - BIG*(k), s) applied ONLY to
the 128 partially-masked diagonal columns of diagonal key-blocks, for both
heads of a pack in one [128,2,128] scalar_tensor_tensor. exp(0.125*masked)
== 0. Fully-masked column ranges (local q < 128r) are skipped entirely:
the scores matmul, exp and ctx matmul all narrow to columns [128r:512].

exp runs on the scalar engine over both heads of a pack at once (the two
heads' score blocks live in one 2-bank PSUM tile), halving per-instruction
overhead. Softmax denominators ride the ctx matmul as an appended
ones-column on V; reciprocals are one batched DVE reciprocal_approx_fast
over a [16,512] tile of all chunk denominators.
"""

import sys
from contextlib import ExitStack

import numpy as np

for _p in ("/opt/trn_rl_repo",):
    if _p not in sys.path:
        sys.path.insert(0, _p)

import ml_dtypes

import concourse.bass as bass
import concourse.tile as tile
from concourse import bacc, mybir
from concourse.bass_utils import run_bass_kernel_spmd

F32 = mybir.dt.float32
BF16 = mybir.dt.bfloat16
U32 = mybir.dt.uint32
AF = mybir.ActivationFunctionType
ALU = mybir.AluOpType

B, S, D = 1, 2048, 2048
H, G, HD = 32, 8, 64
NCORES = 8
HPC = H // NCORES          # 4 q heads per core
BIG = 1000.0
SCW = 512                  # s-chunk width (matmul moving dim)
KBW = 128                  # key block width
LN2 = 0.6931471805599453

# head-dim permutation: new 2j <- old j, new 2j+1 <- old j+32
PERM = np.empty(64, dtype=np.int64)
PERM[0::2] = np.arange(32)
PERM[1::2] = np.arange(32) + 32
PARTNER = np.empty(64, dtype=np.int64)
PARTNER[0::2] = np.arange(1, 64, 2)
PARTNER[1::2] = np.arange(0, 64, 2)
SIGN = np.empty(64, dtype=np.float32)
SIGN[0::2] = -1.0
SIGN[1::2] = 1.0

_SHUF_MASK = [i + 1 if i % 2 == 0 else i - 1 for i in range(32)]


def _rope(nc, pools, psC, bc_ps, n_rows, cg_t, sg_t, out_ap, sc):
    """RoPE + rms-scale on a normalized-projection chunk in SBUF.

    psC: (n_rows, SCW) SBUF f32 raw projected values (d on partitions).
    bc_ps: (n_rows, SCW) PSUM f32 broadcast 1/rms values.
    cg_t/sg_t: (n_rows, S) SBUF cos*g / +-sin*g_partner tables.
    out_ap: destination SBUF AP (n_rows, SCW) bf16.
    """
    tmp = pools["tmp"]
    sl = bass.ts(sc, SCW)
    shuf = tmp.tile([n_rows, SCW], F32, tag="shuf")
    nc.vector.stream_shuffle(shuf, psC, mask=_SHUF_MASK)
    t1 = tmp.tile([n_rows, SCW], F32, tag="t1")
    nc.vector.tensor_mul(t1, psC, cg_t[:, sl])
    t2 = tmp.tile([n_rows, SCW], F32, tag="t2")
    nc.vector.tensor_mul(t2, shuf, sg_t[:, sl])
    t3 = tmp.tile([n_rows, SCW], F32, tag="t3")
    nc.gpsimd.tensor_add(t3, t1, t2)
    nc.vector.tensor_mul(out_ap, t3, bc_ps)


def _build(nc):
    SC = S // SCW
    KB = S // KBW
    DT = D // 128

    B_NAMES = {"xT", "wqT", "wkvT", "woT", "hs5", "bc5", "bcz", "ident"}
    dt_in = {}
    for name, shape in [
        ("xT", [D, S]), ("wqT", [D, 2 * 128]), ("wkvT", [D, 128]),
        ("woT", [2 * 128, D]), ("cgq", [64, S]), ("sgq", [64, S]),
        ("cgk", [64, S]), ("sgk", [64, S]),
        ("hs5", [128, 3, 5]), ("bc5", [5, 3, 128]),
        ("iotaP", [128, 4, 2, 128]), ("thrB", [128, SCW // KBW]),
        ("ident", [64, 64]),
    ]:
        dt_in[name] = nc.dram_tensor(
            name, shape,
            BF16 if name in B_NAMES else F32,
            kind="ExternalInput").ap()
    y_dram = nc.dram_tensor("y", [S, D], BF16, kind="ExternalOutput").ap()

    with tile.TileContext(nc) as tc, ExitStack() as ctx:
        ctx.enter_context(nc.allow_low_precision(
            reason="bf16 matmul operands; fp32 PSUM accumulation throughout"))
        consts = ctx.enter_context(tc.tile_pool(name="consts", bufs=1))
        persist = ctx.enter_context(tc.tile_pool(name="persist", bufs=1))
        tmp = ctx.enter_context(tc.tile_pool(name="tmp", bufs=2))

        # ---- constants / weights into SBUF ----
        def load(name, shape=None, double=False):
            ap = dt_in[name]
            shape = shape or list(ap.shape)
            t = consts.tile(shape, ap.dtype, tag=name, name=name)
            if double:
                nc.sync.dma_start(t[0:64], ap)
                nc.sync.dma_start(t[64:128], ap)
            else:
                nc.sync.dma_start(t, ap.rearrange("(t p) s -> p t s", p=128)
                                  if len(shape) == 3 and name in ("wqT", "wkvT", "woT")
                                  else ap)
            return t

        # projection weights first — phase 1's matmuls gate everything.
        wq_t = load("wqT", [128, DT, 256])
        wkv_t = load("wkvT", [128, DT, 128])


        qrt = [persist.tile([128, S], BF16, tag=f"qrt{p}", name=f"qrt{p}") for p in range(2)]
        krt = persist.tile([128, S], BF16, tag="krt")
        vt = persist.tile([64, S], BF16, tag="vt")
        vaug = persist.tile([128, KB, 65], BF16, tag="vaug")
        ctxT = [persist.tile([128, S], BF16, tag=f"ctxT{p}", name=f"ctxT{p}") for p in range(2)]
        # unnormalized ctx + softmax denominators, consumed in phase 2.5
        ctxU = [[persist.tile([128, SCW], F32, tag=f"ctxU{jc}_{pk}",
                              name=f"ctxU{jc}_{pk}")
                 for pk in range(2)] for jc in range(SC)]
        zall = persist.tile([1, 16 * SCW], F32, tag="zall")

        xT_r = dt_in["xT"].rearrange("(t p) s -> p t s", p=128)

        # ---- fused phase 1+2: projections + norm/rope + attention ----
        # PSUM (8 banks): proj 1x2 + sm 1x2 + sps 2x1 + cps 1x2 = 8.
        # Attention for chunk jc=sc runs right after chunk sc's projections;
        # the scheduler fills attention's exp-wait bubbles with the next
        # chunk's projection matmuls.
        with tc.tile_pool(name="xin", bufs=16) as xin, \
             tc.tile_pool(name="projps", bufs=2, space="PSUM") as projps, \
             tc.tile_pool(name="smallps", bufs=2, space="PSUM") as smallps, \
             tc.tile_pool(name="sps", bufs=1, space="PSUM") as spsp, \
             tc.tile_pool(name="cps", bufs=1, space="PSUM") as cpsp, \
             tc.tile_pool(name="epool", bufs=3) as epool:
            pools = {"tmp": tmp, "smallps": smallps}
            for sc in range(SC):
                if sc == 0:
                    # emit x-chunk DMAs for sc=0 before the table loads so the
                    # first projection matmuls start as early as possible.
                    xts0 = []
                    for dt2 in range(DT // 2):
                        xt = xin.tile([128, 2, SCW], BF16, tag="xt",
                                      name=f"xt0_{dt2}")
                        nc.sync.dma_start(
                            xt, xT_r[:, 2 * dt2:2 * dt2 + 2, bass.ts(0, SCW)])
                        xts0.append(xt)
                    # rope/norm tables — not needed until the first norm.
                    cgq_t = load("cgq", [128, S], double=True)
                    sgq_t = load("sgq", [128, S], double=True)
                    cgk_t = load("cgk")
                    sgk_t = load("sgk")
                    hs5_t = load("hs5")
                    bc5_t = load("bc5")
                    ident_t = load("ident")
                    iota_t = load("iotaP"); thr_t = load("thrB")
                    ones_t = consts.tile([128, KB], BF16, tag="ones",
                                         name="ones")
                    nc.vector.memset(ones_t, 1.0)
                    onesw = consts.tile([1, 64], BF16, tag="onesw",
                                        name="onesw")
                    nc.vector.memset(onesw, 1.0)
                    nc.vector.tensor_copy(
                        out=vaug[:, :, 64:65],
                        in_=ones_t.rearrange("p (k o) -> p k o", o=1))
                ps_list = [projps.tile([128, SCW], F32, tag="proj",
                                       name=f"proj{sc}_{i}") for i in range(3)]
                for dt2 in range(DT // 2):
                    if sc == 0:
                        xt = xts0[dt2]
                    else:
                        xt = xin.tile([128, 2, SCW], BF16, tag="xt")
                        nc.sync.dma_start(
                            xt, xT_r[:, 2 * dt2:2 * dt2 + 2, bass.ts(sc, SCW)])
                    for j in range(2):
                        for i, (w_t, cols) in enumerate(
                                [(wq_t, slice(0, 128)), (wq_t, slice(128, 256)),
                                 (wkv_t, slice(0, 128))]):
                            nc.tensor.matmul(
                                ps_list[i], w_t[:, 2 * dt2 + j, cols],
                                xt[:, j, :],
                                start=(dt2 == 0 and j == 0),
                                stop=(dt2 == DT // 2 - 1 and j == 1))

                # ---- norm: accumulate all 5 mean-square rows into rms5 ----
                rms5 = smallps.tile([5, SCW], F32, tag="sm", name=f"rms5_{sc}")
                psCs = []
                for p in range(2):
                    psC = tmp.tile([128, SCW], F32, tag="psC",
                                   name=f"psC{p}_{sc}")
                    nc.vector.tensor_copy(out=psC, in_=ps_list[p])
                    sq = tmp.tile([128, SCW], BF16, tag="sq")
                    nc.vector.tensor_mul(sq, psC, psC)
                    nc.tensor.matmul(rms5, hs5_t[:, p, :], sq,
                                     start=(p == 0), stop=False)
                    psCs.append(psC)
                kv = ps_list[2]
                nc.vector.tensor_copy(out=vt[:, bass.ts(sc, SCW)], in_=kv[64:128, :])
                psCk = tmp.tile([64, SCW], F32, tag="psC", name=f"psCk_{sc}")
                nc.vector.tensor_copy(out=psCk, in_=kv[0:64, :])
                sqk = tmp.tile([64, SCW], BF16, tag="sq")
                nc.vector.tensor_mul(sqk, psCk, psCk)
                nc.tensor.matmul(rms5, hs5_t[0:64, 2, :], sqk,
                                 start=False, stop=True)

                # ---- rsqrt chain: rcp5 = 1/sqrt(rms5) ----
                # seed = exp(-0.5*ln~(m)) where ln~ is the bit-trick linear
                # log (u32 bits numerically cast to f32, one mul-add); the
                # exp runs on ACT whose table is already resident from the
                # attention exps (zero table loads). One Newton step on DVE
                # polishes to ~1.4e-3, under the bf16 output floor.
                tf5 = tmp.tile([5, SCW], F32, tag="tf5", bufs=1)
                nc.vector.tensor_copy(out=tf5, in_=rms5.bitcast(U32))
                nc.vector.tensor_scalar(
                    out=tf5, in0=tf5, scalar1=float(-0.5 * LN2 / 2.0 ** 23),
                    scalar2=float(0.5 * LN2 * 127.0), op0=ALU.mult,
                    op1=ALU.add)
                y5 = tmp.tile([5, SCW], F32, tag="y5", bufs=1)
                nc.scalar.activation(y5, tf5, AF.Exp)
                a5 = tmp.tile([5, SCW], F32, tag="a5", bufs=1)
                nc.vector.tensor_mul(a5, y5, y5)
                nc.vector.scalar_tensor_tensor(
                    out=a5, in0=a5, scalar=-0.5, in1=rms5,
                    op0=ALU.mult, op1=ALU.mult)
                rcp5 = tmp.tile([5, SCW], BF16, tag="rcp5")
                nc.vector.scalar_tensor_tensor(
                    out=rcp5, in0=a5, scalar=1.5, in1=y5,
                    op0=ALU.add, op1=ALU.mult)

                # ---- broadcast 1/rms + rope ----
                for p in range(2):
                    bc_ps = smallps.tile([128, SCW], F32, tag="sm",
                                         name=f"bcq{p}_{sc}")
                    nc.tensor.matmul(bc_ps, bc5_t[:, p, :], rcp5,
                                     start=True, stop=True)
                    _rope(nc, pools, psCs[p], bc_ps, 128, cgq_t, sgq_t,
                          qrt[p][:, bass.ts(sc, SCW)], sc)
                bc_psk = smallps.tile([64, SCW], F32, tag="sm",
                                      name=f"bck_{sc}")
                nc.tensor.matmul(bc_psk, bc5_t[:, 2, 0:64], rcp5,
                                 start=True, stop=True)
                _rope(nc, pools, psCk, bc_psk, 64, cgk_t, sgk_t,
                      krt[0:64, bass.ts(sc, SCW)], sc)
                # duplicate roped K into rows 64-127 for row-tiled paired scores
                nc.vector.tensor_copy(out=krt[64:128, bass.ts(sc, SCW)],
                                      in_=krt[0:64, bass.ts(sc, SCW)])
                # V transposes for this chunk (tps shares the sm slot family)
                for t in range(sc * 4, sc * 4 + 4):
                    tps = smallps.tile([128, 64], BF16, tag="sm",
                                       name=f"tps{t}")
                    nc.tensor.transpose(tps, vt[:, bass.ts(t, KBW)], ident_t)
                    nc.vector.tensor_copy(out=vaug[:, t, 0:64], in_=tps)
                if sc == 0:
                    wo_t = load("woT", [128, 2, D])  # [p,j,:]=woT[128j+p,:]

                # ---- attention for jc = sc ----
                jc = sc
                nkb = (SCW * (jc + 1)) // KBW
                for pack in range(2):
                    cps = [cpsp.tile([65, SCW], F32, tag=f"cps{h}",
                                     name=f"cps{jc}_{pack}_{h}") for h in range(2)]
                    for kb in range(nkb):
                        r = kb - (SCW // KBW) * jc
                        q0 = 128 * r if r > 0 else 0   # first live local column
                        sps = spsp.tile([128, 2, SCW], F32, tag="sps",
                                        name=f"sps{jc}_{pack}_{kb}")
                        for h in range(2):
                            b = 64 * h
                            nc.tensor.matmul(
                                sps[:, h, q0:], krt[b:b + 64, bass.ts(kb, KBW)],
                                qrt[pack][b:b + 64, jc * SCW + q0:(jc + 1) * SCW],
                                start=True, stop=True, tile_position=(b, 0))
                        if r >= 0:
                            # mask only the 128 partially-masked diagonal
                            # columns, both heads in one op
                            nc.vector.scalar_tensor_tensor(
                                out=sps[:, :, q0:q0 + KBW], in0=iota_t[:, r],
                                scalar=thr_t[:, r:r + 1],
                                in1=sps[:, :, q0:q0 + KBW],
                                op0=ALU.subtract, op1=ALU.min)
                        et = epool.tile([128, 2, SCW], BF16, tag="et")
                        nc.scalar.activation(et[:, :, q0:], sps[:, :, q0:],
                                             AF.Exp, scale=HD ** -0.5)
                        for h in range(2):
                            nc.tensor.matmul(
                                cps[h][:, q0:], vaug[:, kb, :], et[:, h, q0:],
                                start=(kb == 0), stop=(kb == nkb - 1))
                    # stash unnormalized ctx + denominators; normalize later
                    for h in range(2):
                        nc.vector.tensor_copy(
                            out=ctxU[jc][pack][64 * h:64 * h + 64, :],
                            in_=cps[h][0:64, :])
                        zi = (jc * 2 + pack) * 2 + h
                        nc.vector.tensor_copy(
                            out=zall[:, bass.ts(zi, SCW)], in_=cps[h][64:65, :])

        # ---- phase 3: deferred softmax normalization + out-projection ----
        # One pool for the z-broadcast PSUM tiles and the out-proj
        # accumulators: the abc tag allocates first so it takes the two
        # earliest-freed banks (old projection banks) and yps the next
        # four — the scheduler can then hoist normalize + out-projection
        # matmuls of early chunks into the attention tail's exp bubbles.
        with tc.tile_pool(name="yps", bufs=1, space="PSUM") as ypsp, \
             tc.tile_pool(name="yout", bufs=3) as youtp:
            for jc in range(SC):
                zsl = slice(jc * 4 * SCW, (jc + 1) * 4 * SCW)
                nc.vector.reciprocal_approx_fast(out=zall[:, zsl],
                                                 in_=zall[:, zsl])
                zrcp = tmp.tile([1, 4 * SCW], BF16, tag="zrcp",
                                name=f"zrcp{jc}")
                nc.vector.tensor_copy(out=zrcp, in_=zall[:, zsl])
                for pack in range(2):
                    for h in range(2):
                        zi = pack * 2 + h
                        bc_ps = ypsp.tile([64, SCW], F32, tag="abc", bufs=2,
                                          name=f"abc{jc}_{pack}_{h}")
                        nc.tensor.matmul(bc_ps, onesw,
                                         zrcp[:, bass.ts(zi, SCW)],
                                         start=True, stop=True)
                        nc.vector.tensor_mul(
                            ctxT[pack][64 * h:64 * h + 64, bass.ts(jc, SCW)],
                            ctxU[jc][pack][64 * h:64 * h + 64, :], bc_ps)
                for sb in range(4 * jc, 4 * jc + 4):
                    yt = youtp.tile([128, D], BF16, tag="yt")
                    for dc in range(D // SCW):
                        yps = ypsp.tile([128, SCW], F32, tag="yps", bufs=4)
                        for p in range(2):
                            nc.tensor.matmul(
                                yps, ctxT[p][:, bass.ts(sb, 128)],
                                wo_t[:, p, bass.ts(dc, SCW)],
                                start=(p == 0), stop=(p == 1))
                        if dc % 2 == 0:
                            nc.vector.tensor_copy(out=yt[:, bass.ts(dc, SCW)],
                                                  in_=yps)
                        else:
                            nc.scalar.copy(yt[:, bass.ts(dc, SCW)], yps)
                    nc.sync.dma_start(y_dram[bass.ts(sb, 128), :], yt)


_CACHE = {}


def _get_nc():
    if "nc" not in _CACHE:
        nc = bacc.Bacc("TRN2", target_bir_lowering=False, debug=False)
        _build(nc)
        nc.compile()
        _CACHE["nc"] = nc
    return _CACHE["nc"]


def _prep_core_inputs(c, x, cos, sin, wq, wk, wv, wo, qg, kg):
    """Host-side sharding + layout marshaling for core c."""
    f = np.float32
    bf = ml_dtypes.bfloat16
    hsl = slice(c * HPC * HD, (c + 1) * HPC * HD)
    gsl = slice(c * HD, (c + 1) * HD)

    wq_c = wq[hsl, :].reshape(HPC, HD, D)[:, PERM, :].reshape(HPC * HD, D)
    wk_c = wk[gsl, :][PERM, :]
    wv_c = wv[gsl, :]

    cos_p, sin_p = cos[:, PERM], sin[:, PERM]
    qg_p, kg_p = qg[PERM], kg[PERM]

    cgq = np.ascontiguousarray((cos_p * qg_p).T, dtype=f)
    sgq = np.ascontiguousarray((sin_p * (SIGN * qg_p[PARTNER])).T, dtype=f)
    cgk = np.ascontiguousarray((cos_p * kg_p).T, dtype=f)
    sgk = np.ascontiguousarray((sin_p * (SIGN * kg_p[PARTNER])).T, dtype=f)

    # mean-square reduction weights: rms5 row layout
    # rows 0-1: q pack0 heads, rows 2-3: q pack1 heads, row 4: k
    hs5 = np.zeros((128, 3, 5), f)
    hs5[0:64, 0, 0] = 1.0 / HD
    hs5[64:128, 0, 1] = 1.0 / HD
    hs5[0:64, 1, 2] = 1.0 / HD
    hs5[64:128, 1, 3] = 1.0 / HD
    hs5[0:64, 2, 4] = 1.0 / HD
    # broadcast-back weights
    bc5 = np.zeros((5, 3, 128), f)
    bc5[0, 0, 0:64] = 1.0
    bc5[1, 0, 64:128] = 1.0
    bc5[2, 1, 0:64] = 1.0
    bc5[3, 1, 64:128] = 1.0
    bc5[4, 2, 0:64] = 1.0
    iotaP = np.empty((128, 4, 2, 128), f)
    for r in range(4):
        iotaP[:, r, :, :] = ((128 * r + np.arange(128, dtype=f) + 0.5)
                             * BIG)[None, None, :]
    thrB = np.ascontiguousarray(
        (np.arange(128, dtype=f)[:, None]
         + KBW * np.arange(SCW // KBW, dtype=f)[None, :]) * BIG, f)

    return {
        "xT": np.ascontiguousarray(x[0].T).astype(bf),
        "wqT": np.ascontiguousarray(wq_c.T).astype(bf),
        "wkvT": np.ascontiguousarray(np.concatenate([wk_c, wv_c], 0).T).astype(bf),
        "woT": np.ascontiguousarray(wo[:, hsl].T).astype(bf),
        "cgq": cgq, "sgq": sgq, "cgk": cgk, "sgk": sgk,
        "hs5": hs5.astype(bf), "bc5": bc5.astype(bf),
        "iotaP": iotaP, "thrB": thrB,
        "ident": np.eye(64, dtype=f).astype(bf),
    }


def kernel(x, mask, cos, sin, wq, wk, wv, wo, qg, kg, _trace=False):
    nc = _get_nc()
    in_maps = [
        _prep_core_inputs(c, np.asarray(x), np.asarray(cos), np.asarray(sin),
                          np.asarray(wq), np.asarray(wk), np.asarray(wv),
                          np.asarray(wo), np.asarray(qg), np.asarray(kg))
        for c in range(NCORES)
    ]
    res = run_bass_kernel_spmd(nc, in_maps, core_ids=list(range(NCORES)),
                               trace=_trace)
    _CACHE["last_results"] = res
    partials = np.stack([np.asarray(r["y"], dtype=np.float64)
                         for r in res.results], axis=0)
    out = partials.sum(axis=0).astype(np.float32)
    return out.reshape(B, S, D)


# revision 36
# speedup vs baseline: 1.4738x; 1.1673x over previous
"""GQA attention kernel for Trainium2, tensor-parallel over 8 NeuronCores.

Sharding: 4 q-heads + 1 kv-group per core (H=32, G=8). Each core computes
its heads' attention and a partial out-projection; host sums the 8 partials
(the "all-reduce after out_proj").

Device layout strategy: everything transposed (head_dim on partitions,
sequence on the free dim) so Q/K projections, scores and ctx matmuls all
run at moving-dim 512, and no runtime transposes are needed except V
(16 small PE transposes). All heavy matmuls run in bf16 (fp32 PSUM
accumulation): bf16 streams 1 elem/cycle through the PE array vs ~2
cycles for fp32, and LDWEIGHTS gets fast-weight-load.

RMSNorm over head_dim (= partitions) is a ones-matmul partition reduction
for all 5 rows of a chunk (2 q-pack pairs + k) accumulated into ONE [5,512]
PSUM tile, a 5-instruction DVE quake-rsqrt chain (bit-trick seed + one
Newton step; bf16 output floor dominates the seed error), and [5,*]-weight
broadcast-matmuls back to 128/64 partitions. The scalar engine therefore
only ever executes Exp (+Copy), so the ACT table is loaded exactly once —
the Ln/Exp/Reciprocal table-swap thrash (48 x 1.28us) is gone.

RoPE's rotate_half is a 32-lane pairwise stream_shuffle because the
head-dim components are permuted host-side into (i, i+32) pairs
(cos/sin/gains/wq/wk rows permuted to match; wo is NOT permuted since
V/ctx stay in natural order).

Causal mask: scores_masked = min(BIG*(q+0.5) - BIG*(k), s) applied ONLY to
the 128 partially-masked diagonal columns of diagonal key-blocks, for both
heads of a pack in one [128,2,128] scalar_tensor_tensor. exp(0.125*masked)
== 0. Fully-masked column ranges (local q < 128r) are skipped entirely:
the scores matmul, exp and ctx matmul all narrow to columns [128r:512].

exp runs on the scalar engine over both heads of a pack at once (the two
heads' score blocks live in one 2-bank PSUM tile), halving per-instruction
overhead. Softmax denominators ride the ctx matmul as an appended
ones-column on V; reciprocals are one batched DVE reciprocal_approx_fast
over a [16,512] tile of all chunk denominators.
"""

import sys
from contextlib import ExitStack

import numpy as np

for _p in ("/opt/trn_rl_repo",):
    if _p not in sys.path:
        sys.path.insert(0, _p)

import ml_dtypes

import concourse.bass as bass
import concourse.tile as tile
from concourse import bacc, mybir
from concourse.bass_utils import run_bass_kernel_spmd

F32 = mybir.dt.float32
BF16 = mybir.dt.bfloat16
U32 = mybir.dt.uint32
AF = mybir.ActivationFunctionType
ALU = mybir.AluOpType

B, S, D = 1, 2048, 2048
H, G, HD = 32, 8, 64
NCORES = 8
HPC = H // NCORES          # 4 q heads per core
BIG = 1000.0
SCW = 512                  # s-chunk width (matmul moving dim)
KBW = 128                  # key block width
LN2 = 0.6931471805599453

# head-dim permutation: new 2j <- old j, new 2j+1 <- old j+32
PERM = np.empty(64, dtype=np.int64)
PERM[0::2] = np.arange(32)
PERM[1::2] = np.arange(32) + 32
PARTNER = np.empty(64, dtype=np.int64)
PARTNER[0::2] = np.arange(1, 64, 2)
PARTNER[1::2] = np.arange(0, 64, 2)
SIGN = np.empty(64, dtype=np.float32)
SIGN[0::2] = -1.0
SIGN[1::2] = 1.0

_SHUF_MASK = [i + 1 if i % 2 == 0 else i - 1 for i in range(32)]


def _rope(nc, pools, psC, bc_ps, n_rows, cg_t, sg_t, out_ap, sc):
    """RoPE + rms-scale on a normalized-projection chunk in SBUF.

    psC: (n_rows, SCW) SBUF f32 raw projected values (d on partitions).
    bc_ps: (n_rows, SCW) PSUM f32 broadcast 1/rms values.
    cg_t/sg_t: (n_rows, S) SBUF cos*g / +-sin*g_partner tables.
    out_ap: destination SBUF AP (n_rows, SCW) bf16.
    """
    tmp = pools["tmp"]
    sl = bass.ts(sc, SCW)
    shuf = tmp.tile([n_rows, SCW], F32, tag="shuf")
    nc.vector.stream_shuffle(shuf, psC, mask=_SHUF_MASK)
    t1 = tmp.tile([n_rows, SCW], F32, tag="t1")
    nc.vector.tensor_mul(t1, psC, cg_t[:, sl])
    t2 = tmp.tile([n_rows, SCW], F32, tag="t2")
    nc.vector.tensor_mul(t2, shuf, sg_t[:, sl])
    t3 = tmp.tile([n_rows, SCW], F32, tag="t3")
    nc.gpsimd.tensor_add(t3, t1, t2)
    nc.vector.tensor_mul(out_ap, t3, bc_ps)


def _build(nc):
    SC = S // SCW
    KB = S // KBW
    DT = D // 128

    B_NAMES = {"xT", "wqT", "wkvT", "woT", "hs5", "bc5", "bcz", "ident"}
    dt_in = {}
    for name, shape in [
        ("xT", [D, S]), ("wqT", [D, 2 * 128]), ("wkvT", [D, 128]),
        ("woT", [2 * 128, D]), ("cgq", [64, S]), ("sgq", [64, S]),
        ("cgk", [64, S]), ("sgk", [64, S]),
        ("hs5", [128, 3, 5]), ("bc5", [5, 3, 128]),
        ("iotaP", [128, 4, 2, 128]), ("thrB", [128, SCW // KBW]),
        ("ident", [64, 64]),
    ]:
        dt_in[name] = nc.dram_tensor(
            name, shape,
            BF16 if name in B_NAMES else F32,
            kind="ExternalInput").ap()
    y_dram = nc.dram_tensor("y", [S, D], BF16, kind="ExternalOutput").ap()

    with tile.TileContext(nc) as tc, ExitStack() as ctx:
        ctx.enter_context(nc.allow_low_precision(
            reason="bf16 matmul operands; fp32 PSUM accumulation throughout"))
        consts = ctx.enter_context(tc.tile_pool(name="consts", bufs=1))
        persist = ctx.enter_context(tc.tile_pool(name="persist", bufs=1))
        tmp = ctx.enter_context(tc.tile_pool(name="tmp", bufs=2))

        # ---- constants / weights into SBUF ----
        def load(name, shape=None, double=False):
            ap = dt_in[name]
            shape = shape or list(ap.shape)
            t = consts.tile(shape, ap.dtype, tag=name, name=name)
            if double:
                nc.sync.dma_start(t[0:64], ap)
                nc.sync.dma_start(t[64:128], ap)
            else:
                nc.sync.dma_start(t, ap.rearrange("(t p) s -> p t s", p=128)
                                  if len(shape) == 3 and name in ("wqT", "wkvT", "woT")
                                  else ap)
            return t

        # projection weights first — phase 1's matmuls gate everything.
        wq_t = load("wqT", [128, DT, 256])
        wkv_t = load("wkvT", [128, DT, 128])


        qrt = [persist.tile([128, S], BF16, tag=f"qrt{p}", name=f"qrt{p}") for p in range(2)]
        krt = persist.tile([128, S], BF16, tag="krt")
        vt = persist.tile([64, S], BF16, tag="vt")
        vaug = persist.tile([128, KB, 65], BF16, tag="vaug")
        ctxT = [persist.tile([128, S], BF16, tag=f"ctxT{p}", name=f"ctxT{p}") for p in range(2)]
        # unnormalized ctx + softmax denominators, consumed in phase 2.5
        ctxU = [[persist.tile([128, SCW], F32, tag=f"ctxU{jc}_{pk}",
                              name=f"ctxU{jc}_{pk}")
                 for pk in range(2)] for jc in range(SC)]
        zall = persist.tile([1, 16 * SCW], F32, tag="zall")

        xT_r = dt_in["xT"].rearrange("(t p) s -> p t s", p=128)

        # ---- fully fused single scope (pool release = a hard barrier) ----
        # PSUM (8 banks): projfam 1x2 + sps 2x2 + cps 1x2 = 8.
        # ONE rotating 2-bank family ("proj") hosts projections, norm
        # smalls, z-broadcasts AND out-proj accumulators; sps is double-
        # buffered so exp(kb) overlaps the next kb's score matmuls.
        # Out-projection of chunk jc-1 is emitted as a dense block inside
        # chunk jc's norm window, so the PE fills the attention bubbles.
        with tc.tile_pool(name="xin", bufs=10) as xin, \
             tc.tile_pool(name="projps", bufs=2, space="PSUM") as projps, \
             tc.tile_pool(name="sps", bufs=2, space="PSUM") as spsp, \
             tc.tile_pool(name="cps", bufs=1, space="PSUM") as cpsp, \
             tc.tile_pool(name="epool", bufs=3) as epool, \
             tc.tile_pool(name="yout", bufs=2) as youtp:
            pools = {"tmp": tmp}

            def norm_outproj(jc):
                """Deferred softmax normalization + out-projection of chunk
                jc, emitted inside chunk jc+1's norm window."""
                zsl = slice(jc * 4 * SCW, (jc + 1) * 4 * SCW)
                nc.vector.reciprocal_approx_fast(out=zall[:, zsl],
                                                 in_=zall[:, zsl])
                zrcp = tmp.tile([1, 4 * SCW], BF16, tag="zrcp",
                                name=f"zrcp{jc}")
                nc.vector.tensor_copy(out=zrcp, in_=zall[:, zsl])
                for pack in range(2):
                    for h in range(2):
                        zi = pack * 2 + h
                        bc_ps = projps.tile([64, SCW], F32, tag="proj",
                                            name=f"abc{jc}_{pack}_{h}")
                        nc.tensor.matmul(bc_ps, onesw,
                                         zrcp[:, bass.ts(zi, SCW)],
                                         start=True, stop=True)
                        nc.vector.tensor_mul(
                            ctxT[pack][64 * h:64 * h + 64, bass.ts(jc, SCW)],
                            ctxU[jc][pack][64 * h:64 * h + 64, :], bc_ps)
                for sb in range(4 * jc, 4 * jc + 4):
                    yt = youtp.tile([128, D], BF16, tag="yt", name=f"yt{sb}")
                    for dc in range(D // SCW):
                        yps = projps.tile([128, SCW], F32, tag="proj",
                                          name=f"yps{sb}_{dc}")
                        for p in range(2):
                            nc.tensor.matmul(
                                yps, ctxT[p][:, bass.ts(sb, 128)],
                                wo_t[:, p, bass.ts(dc, SCW)],
                                start=(p == 0), stop=(p == 1))
                        if dc % 2 == 0:
                            nc.vector.tensor_copy(out=yt[:, bass.ts(dc, SCW)],
                                                  in_=yps)
                        else:
                            nc.scalar.copy(yt[:, bass.ts(dc, SCW)], yps)
                    nc.sync.dma_start(y_dram[bass.ts(sb, 128), :], yt)

            for sc in range(SC):
                if sc == 0:
                    # emit x-chunk DMAs for sc=0 before the table loads so the
                    # first projection matmuls start as early as possible.
                    xts0 = []
                    for dt2 in range(DT // 2):
                        xt = xin.tile([128, 2, SCW], BF16, tag="xt",
                                      name=f"xt0_{dt2}")
                        nc.sync.dma_start(
                            xt, xT_r[:, 2 * dt2:2 * dt2 + 2, bass.ts(0, SCW)])
                        xts0.append(xt)
                    # rope/norm tables — not needed until the first norm.
                    cgq_t = load("cgq", [128, S], double=True)
                    sgq_t = load("sgq", [128, S], double=True)
                    cgk_t = load("cgk")
                    sgk_t = load("sgk")
                    hs5_t = load("hs5")
                    bc5_t = load("bc5")
                    ident_t = load("ident")
                    iota_t = load("iotaP"); thr_t = load("thrB")
                    ones_t = consts.tile([128, KB], BF16, tag="ones",
                                         name="ones")
                    nc.vector.memset(ones_t, 1.0)
                    onesw = consts.tile([1, 64], BF16, tag="onesw",
                                        name="onesw")
                    nc.vector.memset(onesw, 1.0)
                    nc.vector.tensor_copy(
                        out=vaug[:, :, 64:65],
                        in_=ones_t.rearrange("p (k o) -> p k o", o=1))
                ps_list = [projps.tile([128, SCW], F32, tag="proj",
                                       name=f"proj{sc}_{i}") for i in range(3)]
                for dt2 in range(DT // 2):
                    if sc == 0:
                        xt = xts0[dt2]
                    else:
                        xt = xin.tile([128, 2, SCW], BF16, tag="xt")
                        nc.sync.dma_start(
                            xt, xT_r[:, 2 * dt2:2 * dt2 + 2, bass.ts(sc, SCW)])
                    for j in range(2):
                        for i, (w_t, cols) in enumerate(
                                [(wq_t, slice(0, 128)), (wq_t, slice(128, 256)),
                                 (wkv_t, slice(0, 128))]):
                            nc.tensor.matmul(
                                ps_list[i], w_t[:, 2 * dt2 + j, cols],
                                xt[:, j, :],
                                start=(dt2 == 0 and j == 0),
                                stop=(dt2 == DT // 2 - 1 and j == 1))

                # ---- norm: accumulate all 5 mean-square rows into rms5 ----
                rms5 = projps.tile([5, SCW], F32, tag="proj", name=f"rms5_{sc}")
                psCs = []
                for p in range(2):
                    psC = tmp.tile([128, SCW], F32, tag="psC",
                                   name=f"psC{p}_{sc}")
                    nc.vector.tensor_copy(out=psC, in_=ps_list[p])
                    sq = tmp.tile([128, SCW], BF16, tag="sq")
                    nc.vector.tensor_mul(sq, psC, psC)
                    nc.tensor.matmul(rms5, hs5_t[:, p, :], sq,
                                     start=(p == 0), stop=False)
                    psCs.append(psC)
                kv = ps_list[2]
                nc.vector.tensor_copy(out=vt[:, bass.ts(sc, SCW)], in_=kv[64:128, :])
                psCk = tmp.tile([64, SCW], F32, tag="psC", name=f"psCk_{sc}")
                nc.vector.tensor_copy(out=psCk, in_=kv[0:64, :])
                sqk = tmp.tile([64, SCW], BF16, tag="sq")
                nc.vector.tensor_mul(sqk, psCk, psCk)
                nc.tensor.matmul(rms5, hs5_t[0:64, 2, :], sqk,
                                 start=False, stop=True)

                # ---- rsqrt chain: rcp5 = 1/sqrt(rms5) ----
                # seed = exp(-0.5*ln~(m)) where ln~ is the bit-trick linear
                # log (u32 bits numerically cast to f32, one mul-add); the
                # exp runs on ACT whose table is already resident from the
                # attention exps (zero table loads). One Newton step on DVE
                # polishes to ~1.4e-3, under the bf16 output floor.
                tf5 = tmp.tile([5, SCW], F32, tag="tf5", bufs=1)
                nc.vector.tensor_copy(out=tf5, in_=rms5.bitcast(U32))
                nc.vector.tensor_scalar(
                    out=tf5, in0=tf5, scalar1=float(-0.5 * LN2 / 2.0 ** 23),
                    scalar2=float(0.5 * LN2 * 127.0), op0=ALU.mult,
                    op1=ALU.add)
                y5 = tmp.tile([5, SCW], F32, tag="y5", bufs=1)
                nc.scalar.activation(y5, tf5, AF.Exp)
                a5 = tmp.tile([5, SCW], F32, tag="a5", bufs=1)
                nc.vector.tensor_mul(a5, y5, y5)
                nc.vector.scalar_tensor_tensor(
                    out=a5, in0=a5, scalar=-0.5, in1=rms5,
                    op0=ALU.mult, op1=ALU.mult)
                rcp5 = tmp.tile([5, SCW], BF16, tag="rcp5")
                nc.vector.scalar_tensor_tensor(
                    out=rcp5, in0=a5, scalar=1.5, in1=y5,
                    op0=ALU.add, op1=ALU.mult)

                # ---- broadcast 1/rms + rope ----
                for p in range(2):
                    bc_ps = projps.tile([128, SCW], F32, tag="proj",
                                        name=f"bcq{p}_{sc}")
                    nc.tensor.matmul(bc_ps, bc5_t[:, p, :], rcp5,
                                     start=True, stop=True)
                    _rope(nc, pools, psCs[p], bc_ps, 128, cgq_t, sgq_t,
                          qrt[p][:, bass.ts(sc, SCW)], sc)
                bc_psk = projps.tile([64, SCW], F32, tag="proj",
                                     name=f"bck_{sc}")
                nc.tensor.matmul(bc_psk, bc5_t[:, 2, 0:64], rcp5,
                                 start=True, stop=True)
                _rope(nc, pools, psCk, bc_psk, 64, cgk_t, sgk_t,
                      krt[0:64, bass.ts(sc, SCW)], sc)
                # duplicate roped K into rows 64-127 for row-tiled paired scores
                nc.vector.tensor_copy(out=krt[64:128, bass.ts(sc, SCW)],
                                      in_=krt[0:64, bass.ts(sc, SCW)])
                # V transposes for this chunk (tps shares the proj family)
                for t in range(sc * 4, sc * 4 + 4):
                    tps = projps.tile([128, 64], BF16, tag="proj",
                                      name=f"tps{t}")
                    nc.tensor.transpose(tps, vt[:, bass.ts(t, KBW)], ident_t)
                    nc.vector.tensor_copy(out=vaug[:, t, 0:64], in_=tps)
                if sc == 0:
                    wo_t = load("woT", [128, 2, D])  # [p,j,:]=woT[128j+p,:]

                # previous chunk's normalize + out-projection: fills the PE
                # while this chunk's norm chain runs on DVE/ACT
                if sc > 0:
                    norm_outproj(sc - 1)

                # ---- attention for jc = sc ----
                jc = sc
                nkb = (SCW * (jc + 1)) // KBW
                for pack in range(2):
                    cps = [cpsp.tile([65, SCW], F32, tag=f"cps{h}",
                                     name=f"cps{jc}_{pack}_{h}") for h in range(2)]
                    for kb in range(nkb):
                        r = kb - (SCW // KBW) * jc
                        q0 = 128 * r if r > 0 else 0   # first live local column
                        sps = spsp.tile([128, 2, SCW], F32, tag="sps",
                                        name=f"sps{jc}_{pack}_{kb}")
                        for h in range(2):
                            b = 64 * h
                            nc.tensor.matmul(
                                sps[:, h, q0:], krt[b:b + 64, bass.ts(kb, KBW)],
                                qrt[pack][b:b + 64, jc * SCW + q0:(jc + 1) * SCW],
                                start=True, stop=True, tile_position=(b, 0))
                        if r >= 0:
                            # mask only the 128 partially-masked diagonal
                            # columns, both heads in one op
                            nc.vector.scalar_tensor_tensor(
                                out=sps[:, :, q0:q0 + KBW], in0=iota_t[:, r],
                                scalar=thr_t[:, r:r + 1],
                                in1=sps[:, :, q0:q0 + KBW],
                                op0=ALU.subtract, op1=ALU.min)
                        et = epool.tile([128, 2, SCW], BF16, tag="et")
                        nc.scalar.activation(et[:, :, q0:], sps[:, :, q0:],
                                             AF.Exp, scale=HD ** -0.5)
                        for h in range(2):
                            nc.tensor.matmul(
                                cps[h][:, q0:], vaug[:, kb, :], et[:, h, q0:],
                                start=(kb == 0), stop=(kb == nkb - 1))
                    # stash unnormalized ctx + denominators; normalize later
                    for h in range(2):
                        nc.vector.tensor_copy(
                            out=ctxU[jc][pack][64 * h:64 * h + 64, :],
                            in_=cps[h][0:64, :])
                        zi = (jc * 2 + pack) * 2 + h
                        nc.vector.tensor_copy(
                            out=zall[:, bass.ts(zi, SCW)], in_=cps[h][64:65, :])
            # last chunk's normalize + out-projection
            norm_outproj(SC - 1)


_CACHE = {}


def _get_nc():
    if "nc" not in _CACHE:
        nc = bacc.Bacc("TRN2", target_bir_lowering=False, debug=False)
        _build(nc)
        nc.compile()
        _CACHE["nc"] = nc
    return _CACHE["nc"]


def _prep_core_inputs(c, x, cos, sin, wq, wk, wv, wo, qg, kg):
    """Host-side sharding + layout marshaling for core c."""
    f = np.float32
    bf = ml_dtypes.bfloat16
    hsl = slice(c * HPC * HD, (c + 1) * HPC * HD)
    gsl = slice(c * HD, (c + 1) * HD)

    wq_c = wq[hsl, :].reshape(HPC, HD, D)[:, PERM, :].reshape(HPC * HD, D)
    wk_c = wk[gsl, :][PERM, :]
    wv_c = wv[gsl, :]

    cos_p, sin_p = cos[:, PERM], sin[:, PERM]
    qg_p, kg_p = qg[PERM], kg[PERM]

    cgq = np.ascontiguousarray((cos_p * qg_p).T, dtype=f)
    sgq = np.ascontiguousarray((sin_p * (SIGN * qg_p[PARTNER])).T, dtype=f)
    cgk = np.ascontiguousarray((cos_p * kg_p).T, dtype=f)
    sgk = np.ascontiguousarray((sin_p * (SIGN * kg_p[PARTNER])).T, dtype=f)

    # mean-square reduction weights: rms5 row layout
    # rows 0-1: q pack0 heads, rows 2-3: q pack1 heads, row 4: k
    hs5 = np.zeros((128, 3, 5), f)
    hs5[0:64, 0, 0] = 1.0 / HD
    hs5[64:128, 0, 1] = 1.0 / HD
    hs5[0:64, 1, 2] = 1.0 / HD
    hs5[64:128, 1, 3] = 1.0 / HD
    hs5[0:64, 2, 4] = 1.0 / HD
    # broadcast-back weights
    bc5 = np.zeros((5, 3, 128), f)
    bc5[0, 0, 0:64] = 1.0
    bc5[1, 0, 64:128] = 1.0
    bc5[2, 1, 0:64] = 1.0
    bc5[3, 1, 64:128] = 1.0
    bc5[4, 2, 0:64] = 1.0
    iotaP = np.empty((128, 4, 2, 128), f)
    for r in range(4):
        iotaP[:, r, :, :] = ((128 * r + np.arange(128, dtype=f) + 0.5)
                             * BIG)[None, None, :]
    thrB = np.ascontiguousarray(
        (np.arange(128, dtype=f)[:, None]
         + KBW * np.arange(SCW // KBW, dtype=f)[None, :]) * BIG, f)

    return {
        "xT": np.ascontiguousarray(x[0].T).astype(bf),
        "wqT": np.ascontiguousarray(wq_c.T).astype(bf),
        "wkvT": np.ascontiguousarray(np.concatenate([wk_c, wv_c], 0).T).astype(bf),
        "woT": np.ascontiguousarray(wo[:, hsl].T).astype(bf),
        "cgq": cgq, "sgq": sgq, "cgk": cgk, "sgk": sgk,
        "hs5": hs5.astype(bf), "bc5": bc5.astype(bf),
        "iotaP": iotaP, "thrB": thrB,
        "ident": np.eye(64, dtype=f).astype(bf),
    }


def kernel(x, mask, cos, sin, wq, wk, wv, wo, qg, kg, _trace=False):
    nc = _get_nc()
    in_maps = [
        _prep_core_inputs(c, np.asarray(x), np.asarray(cos), np.asarray(sin),
                          np.asarray(wq), np.asarray(wk), np.asarray(wv),
                          np.asarray(wo), np.asarray(qg), np.asarray(kg))
        for c in range(NCORES)
    ]
    res = run_bass_kernel_spmd(nc, in_maps, core_ids=list(range(NCORES)),
                               trace=_trace)
    _CACHE["last_results"] = res
    partials = np.stack([np.asarray(r["y"], dtype=np.float64)
                         for r in res.results], axis=0)
    out = partials.sum(axis=0).astype(np.float32)
    return out.reshape(B, S, D)


# revision 42
# speedup vs baseline: 1.4761x; 1.0016x over previous
"""GQA attention kernel for Trainium2, tensor-parallel over 8 NeuronCores.

Sharding: 4 q-heads + 1 kv-group per core (H=32, G=8). Each core computes
its heads' attention and a partial out-projection; host sums the 8 partials
(the "all-reduce after out_proj").

Device layout strategy: everything transposed (head_dim on partitions,
sequence on the free dim) so Q/K projections, scores and ctx matmuls all
run at moving-dim 512, and no runtime transposes are needed except V
(16 small PE transposes). All heavy matmuls run in bf16 (fp32 PSUM
accumulation): bf16 streams 1 elem/cycle through the PE array vs ~2
cycles for fp32, and LDWEIGHTS gets fast-weight-load.

RMSNorm over head_dim (= partitions) is a ones-matmul partition reduction
for all 5 rows of a chunk (2 q-pack pairs + k) accumulated into ONE [5,512]
PSUM tile, a 5-instruction DVE quake-rsqrt chain (bit-trick seed + one
Newton step; bf16 output floor dominates the seed error), and [5,*]-weight
broadcast-matmuls back to 128/64 partitions. The scalar engine therefore
only ever executes Exp (+Copy), so the ACT table is loaded exactly once —
the Ln/Exp/Reciprocal table-swap thrash (48 x 1.28us) is gone.

RoPE's rotate_half is a 32-lane pairwise stream_shuffle because the
head-dim components are permuted host-side into (i, i+32) pairs
(cos/sin/gains/wq/wk rows permuted to match; wo is NOT permuted since
V/ctx stay in natural order).

Causal mask: scores_masked = min(BIG*(q+0.5) - BIG*(k), s) applied ONLY to
the 128 partially-masked diagonal columns of diagonal key-blocks, for both
heads of a pack in one [128,2,128] scalar_tensor_tensor. exp(0.125*masked)
== 0. Fully-masked column ranges (local q < 128r) are skipped entirely:
the scores matmul, exp and ctx matmul all narrow to columns [128r:512].

exp runs on the scalar engine over both heads of a pack at once (the two
heads' score blocks live in one 2-bank PSUM tile), halving per-instruction
overhead. Softmax denominators ride the ctx matmul as an appended
ones-column on V; reciprocals are one batched DVE reciprocal_approx_fast
over a [16,512] tile of all chunk denominators.
"""

import sys
from contextlib import ExitStack

import numpy as np

for _p in ("/opt/trn_rl_repo",):
    if _p not in sys.path:
        sys.path.insert(0, _p)

import ml_dtypes

import concourse.bass as bass
import concourse.tile as tile
from concourse import bacc, mybir
from concourse.bass_utils import run_bass_kernel_spmd

F32 = mybir.dt.float32
BF16 = mybir.dt.bfloat16
U32 = mybir.dt.uint32
AF = mybir.ActivationFunctionType
ALU = mybir.AluOpType

B, S, D = 1, 2048, 2048
H, G, HD = 32, 8, 64
NCORES = 8
HPC = H // NCORES          # 4 q heads per core
BIG = 1000.0
SCW = 512                  # s-chunk width (matmul moving dim)
KBW = 128                  # key block width
LN2 = 0.6931471805599453

# head-dim permutation: new 2j <- old j, new 2j+1 <- old j+32
PERM = np.empty(64, dtype=np.int64)
PERM[0::2] = np.arange(32)
PERM[1::2] = np.arange(32) + 32
PARTNER = np.empty(64, dtype=np.int64)
PARTNER[0::2] = np.arange(1, 64, 2)
PARTNER[1::2] = np.arange(0, 64, 2)
SIGN = np.empty(64, dtype=np.float32)
SIGN[0::2] = -1.0
SIGN[1::2] = 1.0

_SHUF_MASK = [i + 1 if i % 2 == 0 else i - 1 for i in range(32)]


def _rope(nc, pools, psC, bc_ps, n_rows, cg_t, sg_t, out_ap, sc):
    """RoPE + rms-scale on a normalized-projection chunk in SBUF.

    psC: (n_rows, SCW) SBUF f32 raw projected values (d on partitions).
    bc_ps: (n_rows, SCW) PSUM f32 broadcast 1/rms values.
    cg_t/sg_t: (n_rows, S) SBUF cos*g / +-sin*g_partner tables.
    out_ap: destination SBUF AP (n_rows, SCW) bf16.
    """
    tmp = pools["tmp"]
    sl = bass.ts(sc, SCW)
    shuf = tmp.tile([n_rows, SCW], F32, tag="shuf")
    nc.vector.stream_shuffle(shuf, psC, mask=_SHUF_MASK)
    t1 = tmp.tile([n_rows, SCW], F32, tag="t1")
    nc.vector.tensor_mul(t1, psC, cg_t[:, sl])
    t2 = tmp.tile([n_rows, SCW], F32, tag="t2")
    nc.vector.tensor_mul(t2, shuf, sg_t[:, sl])
    t3 = tmp.tile([n_rows, SCW], F32, tag="t3")
    nc.gpsimd.tensor_add(t3, t1, t2)
    nc.vector.tensor_mul(out_ap, t3, bc_ps)


def _build(nc):
    SC = S // SCW
    KB = S // KBW
    DT = D // 128

    B_NAMES = {"xT", "wqT", "wkvT", "woT", "hs5", "bc5", "bcz", "ident"}
    dt_in = {}
    for name, shape in [
        ("xT", [D, S]), ("wqT", [D, 2 * 128]), ("wkvT", [D, 128]),
        ("woT", [2 * 128, D]), ("cgq", [64, S]), ("sgq", [64, S]),
        ("cgk", [64, S]), ("sgk", [64, S]),
        ("hs5", [128, 3, 5]), ("bc5", [5, 3, 128]),
        ("iotaP", [128, 4, 2, 128]), ("thrB", [128, SCW // KBW]),
        ("ident", [64, 64]),
    ]:
        dt_in[name] = nc.dram_tensor(
            name, shape,
            BF16 if name in B_NAMES else F32,
            kind="ExternalInput").ap()
    y_dram = nc.dram_tensor("y", [S, D], BF16, kind="ExternalOutput").ap()

    with tile.TileContext(nc) as tc, ExitStack() as ctx:
        ctx.enter_context(nc.allow_low_precision(
            reason="bf16 matmul operands; fp32 PSUM accumulation throughout"))
        consts = ctx.enter_context(tc.tile_pool(name="consts", bufs=1))
        persist = ctx.enter_context(tc.tile_pool(name="persist", bufs=1))
        tmp = ctx.enter_context(tc.tile_pool(name="tmp", bufs=2))

        # ---- constants / weights into SBUF ----
        def load(name, shape=None, double=False):
            ap = dt_in[name]
            shape = shape or list(ap.shape)
            t = consts.tile(shape, ap.dtype, tag=name, name=name)
            if double:
                nc.sync.dma_start(t[0:64], ap)
                nc.sync.dma_start(t[64:128], ap)
            else:
                nc.sync.dma_start(t, ap.rearrange("(t p) s -> p t s", p=128)
                                  if len(shape) == 3 and name in ("wqT", "wkvT", "woT")
                                  else ap)
            return t

        # projection weights first — phase 1's matmuls gate everything.
        wq_t = load("wqT", [128, DT, 256])
        wkv_t = load("wkvT", [128, DT, 128])


        qrt = [persist.tile([128, S], BF16, tag=f"qrt{p}", name=f"qrt{p}") for p in range(2)]
        krt = persist.tile([128, S], BF16, tag="krt")
        vt = persist.tile([64, S], BF16, tag="vt")
        vaug = persist.tile([128, KB, 65], BF16, tag="vaug")
        ctxT = [persist.tile([128, S], BF16, tag=f"ctxT{p}", name=f"ctxT{p}") for p in range(2)]
        # unnormalized ctx + softmax denominators, consumed in phase 2.5
        ctxU = [[persist.tile([128, SCW], F32, tag=f"ctxU{jc}_{pk}",
                              name=f"ctxU{jc}_{pk}")
                 for pk in range(2)] for jc in range(SC)]
        zall = persist.tile([1, 16 * SCW], F32, tag="zall")

        xT_r = dt_in["xT"].rearrange("(t p) s -> p t s", p=128)

        # ---- fully fused single scope (pool release = a hard barrier) ----
        # PSUM (8 banks): projfam 1x2 + sps 2x2 + cps 1x2 = 8.
        # ONE rotating 2-bank family ("proj") hosts projections, norm
        # smalls, z-broadcasts AND out-proj accumulators; sps is double-
        # buffered so exp(kb) overlaps the next kb's score matmuls.
        # Out-projection of chunk jc-1 is emitted as a dense block inside
        # chunk jc's norm window, so the PE fills the attention bubbles.
        with tc.tile_pool(name="xin", bufs=10) as xin, \
             tc.tile_pool(name="projps", bufs=2, space="PSUM") as projps, \
             tc.tile_pool(name="sps", bufs=2, space="PSUM") as spsp, \
             tc.tile_pool(name="cps", bufs=1, space="PSUM") as cpsp, \
             tc.tile_pool(name="epool", bufs=3) as epool, \
             tc.tile_pool(name="yout", bufs=2) as youtp:
            pools = {"tmp": tmp}

            def norm_outproj(jc):
                """Deferred softmax normalization + out-projection of chunk
                jc, emitted inside chunk jc+1's norm window."""
                zsl = slice(jc * 4 * SCW, (jc + 1) * 4 * SCW)
                nc.vector.reciprocal_approx_fast(out=zall[:, zsl],
                                                 in_=zall[:, zsl])
                zrcp = tmp.tile([1, 4 * SCW], BF16, tag="zrcp",
                                name=f"zrcp{jc}")
                nc.vector.tensor_copy(out=zrcp, in_=zall[:, zsl])
                for pack in range(2):
                    for h in range(2):
                        zi = pack * 2 + h
                        bc_ps = projps.tile([64, SCW], F32, tag="proj",
                                            name=f"abc{jc}_{pack}_{h}")
                        nc.tensor.matmul(bc_ps, onesw,
                                         zrcp[:, bass.ts(zi, SCW)],
                                         start=True, stop=True)
                        nc.vector.tensor_mul(
                            ctxT[pack][64 * h:64 * h + 64, bass.ts(jc, SCW)],
                            ctxU[jc][pack][64 * h:64 * h + 64, :], bc_ps)
                for sb in range(4 * jc, 4 * jc + 4):
                    yt = youtp.tile([128, D], BF16, tag="yt", name=f"yt{sb}")
                    for dc in range(D // SCW):
                        yps = projps.tile([128, SCW], F32, tag="proj",
                                          name=f"yps{sb}_{dc}")
                        for p in range(2):
                            nc.tensor.matmul(
                                yps, ctxT[p][:, bass.ts(sb, 128)],
                                wo_t[:, p, bass.ts(dc, SCW)],
                                start=(p == 0), stop=(p == 1))
                        if dc == 2:
                            nc.vector.tensor_copy(
                                out=yt[:, bass.ts(dc, SCW)], in_=yps)
                        else:
                            nc.scalar.copy(yt[:, bass.ts(dc, SCW)], yps)
                    nc.sync.dma_start(y_dram[bass.ts(sb, 128), :], yt)

            for sc in range(SC):
                if sc == 0:
                    # emit x-chunk DMAs for sc=0 before the table loads so the
                    # first projection matmuls start as early as possible.
                    xts0 = []
                    for dt2 in range(DT // 2):
                        xt = xin.tile([128, 2, SCW], BF16, tag="xt",
                                      name=f"xt0_{dt2}")
                        nc.sync.dma_start(
                            xt, xT_r[:, 2 * dt2:2 * dt2 + 2, bass.ts(0, SCW)])
                        xts0.append(xt)
                    # rope/norm tables — not needed until the first norm.
                    cgq_t = load("cgq", [128, S], double=True)
                    sgq_t = load("sgq", [128, S], double=True)
                    cgk_t = load("cgk")
                    sgk_t = load("sgk")
                    hs5_t = load("hs5")
                    bc5_t = load("bc5")
                    ident_t = load("ident")
                    iota_t = load("iotaP"); thr_t = load("thrB")
                    ones_t = consts.tile([128, KB], BF16, tag="ones",
                                         name="ones")
                    nc.vector.memset(ones_t, 1.0)
                    onesw = consts.tile([1, 64], BF16, tag="onesw",
                                        name="onesw")
                    nc.vector.memset(onesw, 1.0)
                    nc.vector.tensor_copy(
                        out=vaug[:, :, 64:65],
                        in_=ones_t.rearrange("p (k o) -> p k o", o=1))
                ps_list = [projps.tile([128, SCW], F32, tag="proj",
                                       name=f"proj{sc}_{i}") for i in range(3)]
                for dt2 in range(DT // 2):
                    if sc == 0:
                        xt = xts0[dt2]
                    else:
                        xt = xin.tile([128, 2, SCW], BF16, tag="xt")
                        nc.sync.dma_start(
                            xt, xT_r[:, 2 * dt2:2 * dt2 + 2, bass.ts(sc, SCW)])
                    for j in range(2):
                        for i, (w_t, cols) in enumerate(
                                [(wq_t, slice(0, 128)), (wq_t, slice(128, 256)),
                                 (wkv_t, slice(0, 128))]):
                            nc.tensor.matmul(
                                ps_list[i], w_t[:, 2 * dt2 + j, cols],
                                xt[:, j, :],
                                start=(dt2 == 0 and j == 0),
                                stop=(dt2 == DT // 2 - 1 and j == 1))

                # ---- norm: accumulate all 5 mean-square rows into rms5 ----
                # squares run on ACT (Square lives in every table set) and
                # read the projection PSUM directly, in parallel with the
                # DVE drain copies — the chunk-boundary DVE convoy was the
                # critical path.
                rms5 = projps.tile([5, SCW], F32, tag="proj", name=f"rms5_{sc}")
                psCs = []
                for p in range(2):
                    psC = tmp.tile([128, SCW], F32, tag="psC",
                                   name=f"psC{p}_{sc}")
                    if p == 0:
                        nc.vector.tensor_copy(out=psC, in_=ps_list[p])
                    else:
                        nc.scalar.copy(psC, ps_list[p])
                    sq = tmp.tile([128, SCW], BF16, tag="sq")
                    nc.scalar.activation(sq, ps_list[p], AF.Square)
                    nc.tensor.matmul(rms5, hs5_t[:, p, :], sq,
                                     start=(p == 0), stop=False)
                    psCs.append(psC)
                kv = ps_list[2]
                nc.scalar.copy(vt[:, bass.ts(sc, SCW)], kv[64:128, :])
                psCk = tmp.tile([64, SCW], F32, tag="psC", name=f"psCk_{sc}")
                nc.vector.tensor_copy(out=psCk, in_=kv[0:64, :])
                sqk = tmp.tile([64, SCW], BF16, tag="sq")
                nc.scalar.activation(sqk, kv[0:64, :], AF.Square)
                nc.tensor.matmul(rms5, hs5_t[0:64, 2, :], sqk,
                                 start=False, stop=True)

                # ---- rsqrt chain: rcp5 = 1/sqrt(rms5) ----
                # seed = exp(-0.5*ln~(m)) where ln~ is the bit-trick linear
                # log (u32 bits numerically cast to f32, one mul-add); the
                # exp runs on ACT whose table is already resident from the
                # attention exps (zero table loads). One Newton step on DVE
                # polishes to ~1.4e-3, under the bf16 output floor.
                tf5 = tmp.tile([5, SCW], F32, tag="tf5", bufs=1)
                nc.vector.tensor_copy(out=tf5, in_=rms5.bitcast(U32))
                nc.vector.tensor_scalar(
                    out=tf5, in0=tf5, scalar1=float(-0.5 * LN2 / 2.0 ** 23),
                    scalar2=float(0.5 * LN2 * 127.0), op0=ALU.mult,
                    op1=ALU.add)
                y5 = tmp.tile([5, SCW], F32, tag="y5", bufs=1)
                nc.scalar.activation(y5, tf5, AF.Exp)
                a5 = tmp.tile([5, SCW], F32, tag="a5", bufs=1)
                nc.scalar.activation(a5, y5, AF.Square)
                nc.vector.scalar_tensor_tensor(
                    out=a5, in0=a5, scalar=-0.5, in1=rms5,
                    op0=ALU.mult, op1=ALU.mult)
                rcp5 = tmp.tile([5, SCW], BF16, tag="rcp5")
                nc.vector.scalar_tensor_tensor(
                    out=rcp5, in0=a5, scalar=1.5, in1=y5,
                    op0=ALU.add, op1=ALU.mult)

                # ---- broadcast 1/rms + rope ----
                for p in range(2):
                    bc_ps = projps.tile([128, SCW], F32, tag="proj",
                                        name=f"bcq{p}_{sc}")
                    nc.tensor.matmul(bc_ps, bc5_t[:, p, :], rcp5,
                                     start=True, stop=True)
                    _rope(nc, pools, psCs[p], bc_ps, 128, cgq_t, sgq_t,
                          qrt[p][:, bass.ts(sc, SCW)], sc)
                bc_psk = projps.tile([64, SCW], F32, tag="proj",
                                     name=f"bck_{sc}")
                nc.tensor.matmul(bc_psk, bc5_t[:, 2, 0:64], rcp5,
                                 start=True, stop=True)
                _rope(nc, pools, psCk, bc_psk, 64, cgk_t, sgk_t,
                      krt[0:64, bass.ts(sc, SCW)], sc)
                # duplicate roped K into rows 64-127 for row-tiled paired scores
                nc.scalar.copy(krt[64:128, bass.ts(sc, SCW)],
                               krt[0:64, bass.ts(sc, SCW)])
                # V transposes for this chunk (tps shares the proj family)
                for t in range(sc * 4, sc * 4 + 4):
                    tps = projps.tile([128, 64], BF16, tag="proj",
                                      name=f"tps{t}")
                    nc.tensor.transpose(tps, vt[:, bass.ts(t, KBW)], ident_t)
                    nc.vector.tensor_copy(out=vaug[:, t, 0:64], in_=tps)
                if sc == 0:
                    wo_t = load("woT", [128, 2, D])  # [p,j,:]=woT[128j+p,:]

                # previous chunk's normalize + out-projection: fills the PE
                # while this chunk's norm chain runs on DVE/ACT
                if sc > 0:
                    norm_outproj(sc - 1)

                # ---- attention for jc = sc ----
                jc = sc
                nkb = (SCW * (jc + 1)) // KBW
                for pack in range(2):
                    cps = [cpsp.tile([65, SCW], F32, tag=f"cps{h}",
                                     name=f"cps{jc}_{pack}_{h}") for h in range(2)]
                    for kb in range(nkb):
                        r = kb - (SCW // KBW) * jc
                        q0 = 128 * r if r > 0 else 0   # first live local column
                        sps = spsp.tile([128, 2, SCW], F32, tag="sps",
                                        name=f"sps{jc}_{pack}_{kb}")
                        for h in range(2):
                            b = 64 * h
                            nc.tensor.matmul(
                                sps[:, h, q0:], krt[b:b + 64, bass.ts(kb, KBW)],
                                qrt[pack][b:b + 64, jc * SCW + q0:(jc + 1) * SCW],
                                start=True, stop=True, tile_position=(b, 0))
                        if r >= 0:
                            # mask only the 128 partially-masked diagonal
                            # columns, both heads in one op
                            nc.vector.scalar_tensor_tensor(
                                out=sps[:, :, q0:q0 + KBW], in0=iota_t[:, r],
                                scalar=thr_t[:, r:r + 1],
                                in1=sps[:, :, q0:q0 + KBW],
                                op0=ALU.subtract, op1=ALU.min)
                        et = epool.tile([128, 2, SCW], BF16, tag="et")
                        nc.scalar.activation(et[:, :, q0:], sps[:, :, q0:],
                                             AF.Exp, scale=HD ** -0.5)
                        for h in range(2):
                            nc.tensor.matmul(
                                cps[h][:, q0:], vaug[:, kb, :], et[:, h, q0:],
                                start=(kb == 0), stop=(kb == nkb - 1))
                    # stash unnormalized ctx + denominators; normalize later
                    for h in range(2):
                        if h == 0:
                            nc.scalar.copy(
                                ctxU[jc][pack][0:64, :], cps[h][0:64, :])
                        else:
                            nc.vector.tensor_copy(
                                out=ctxU[jc][pack][64:128, :],
                                in_=cps[h][0:64, :])
                        zi = (jc * 2 + pack) * 2 + h
                        nc.vector.tensor_copy(
                            out=zall[:, bass.ts(zi, SCW)], in_=cps[h][64:65, :])
            # last chunk's normalize + out-projection
            norm_outproj(SC - 1)


_CACHE = {}


def _get_nc():
    if "nc" not in _CACHE:
        nc = bacc.Bacc("TRN2", target_bir_lowering=False, debug=False)
        _build(nc)
        nc.compile()
        _CACHE["nc"] = nc
    return _CACHE["nc"]


def _prep_core_inputs(c, x, cos, sin, wq, wk, wv, wo, qg, kg):
    """Host-side sharding + layout marshaling for core c."""
    f = np.float32
    bf = ml_dtypes.bfloat16
    hsl = slice(c * HPC * HD, (c + 1) * HPC * HD)
    gsl = slice(c * HD, (c + 1) * HD)

    wq_c = wq[hsl, :].reshape(HPC, HD, D)[:, PERM, :].reshape(HPC * HD, D)
    wk_c = wk[gsl, :][PERM, :]
    wv_c = wv[gsl, :]

    cos_p, sin_p = cos[:, PERM], sin[:, PERM]
    qg_p, kg_p = qg[PERM], kg[PERM]

    cgq = np.ascontiguousarray((cos_p * qg_p).T, dtype=f)
    sgq = np.ascontiguousarray((sin_p * (SIGN * qg_p[PARTNER])).T, dtype=f)
    cgk = np.ascontiguousarray((cos_p * kg_p).T, dtype=f)
    sgk = np.ascontiguousarray((sin_p * (SIGN * kg_p[PARTNER])).T, dtype=f)

    # mean-square reduction weights: rms5 row layout
    # rows 0-1: q pack0 heads, rows 2-3: q pack1 heads, row 4: k
    hs5 = np.zeros((128, 3, 5), f)
    hs5[0:64, 0, 0] = 1.0 / HD
    hs5[64:128, 0, 1] = 1.0 / HD
    hs5[0:64, 1, 2] = 1.0 / HD
    hs5[64:128, 1, 3] = 1.0 / HD
    hs5[0:64, 2, 4] = 1.0 / HD
    # broadcast-back weights
    bc5 = np.zeros((5, 3, 128), f)
    bc5[0, 0, 0:64] = 1.0
    bc5[1, 0, 64:128] = 1.0
    bc5[2, 1, 0:64] = 1.0
    bc5[3, 1, 64:128] = 1.0
    bc5[4, 2, 0:64] = 1.0
    iotaP = np.empty((128, 4, 2, 128), f)
    for r in range(4):
        iotaP[:, r, :, :] = ((128 * r + np.arange(128, dtype=f) + 0.5)
                             * BIG)[None, None, :]
    thrB = np.ascontiguousarray(
        (np.arange(128, dtype=f)[:, None]
         + KBW * np.arange(SCW // KBW, dtype=f)[None, :]) * BIG, f)

    return {
        "xT": np.ascontiguousarray(x[0].T).astype(bf),
        "wqT": np.ascontiguousarray(wq_c.T).astype(bf),
        "wkvT": np.ascontiguousarray(np.concatenate([wk_c, wv_c], 0).T).astype(bf),
        "woT": np.ascontiguousarray(wo[:, hsl].T).astype(bf),
        "cgq": cgq, "sgq": sgq, "cgk": cgk, "sgk": sgk,
        "hs5": hs5.astype(bf), "bc5": bc5.astype(bf),
        "iotaP": iotaP, "thrB": thrB,
        "ident": np.eye(64, dtype=f).astype(bf),
    }


def kernel(x, mask, cos, sin, wq, wk, wv, wo, qg, kg, _trace=False):
    nc = _get_nc()
    in_maps = [
        _prep_core_inputs(c, np.asarray(x), np.asarray(cos), np.asarray(sin),
                          np.asarray(wq), np.asarray(wk), np.asarray(wv),
                          np.asarray(wo), np.asarray(qg), np.asarray(kg))
        for c in range(NCORES)
    ]
    res = run_bass_kernel_spmd(nc, in_maps, core_ids=list(range(NCORES)),
                               trace=_trace)
    _CACHE["last_results"] = res
    partials = np.stack([np.asarray(r["y"], dtype=np.float64)
                         for r in res.results], axis=0)
    out = partials.sum(axis=0).astype(np.float32)
    return out.reshape(B, S, D)
